# revision 1
# baseline (speedup 1.0000x reference)
"""Trainium2 Bass kernel for nn_ACMnnAttnEmb (Longformer-style, S=4096, 12 layers).

Sequence-parallel over 8 NeuronCores: 512 tokens (2 attention blocks) per core.
Per layer: one bf16 AllGather of the residual stream (halo + global token) and
one tiny f32 AllReduce for the global-attention row. All matmuls bf16 with f32
PSUM accumulation; LayerNorm/softmax in f32. Softmax is computed without
max-subtraction (scores empirically bounded |s| < 3) with denominators folded
into the PV matmul via a ones-column in V.

Self-contained: shapes/sharding hardcoded; host pre-packs weight tiles, band
masks, halo gather indices, and the embedding-row gather.
"""
import numpy as np
from ml_dtypes import bfloat16

NCORE = 8
S = 4096; D = 768; H = 12; DH = 64; F = 3072; L = 12
BLK = 256; NB = S // BLK
TOK = S // NCORE          # 512 tokens per core
HT = TOK + 2 * BLK        # 1024-token halo window
DT = D // 128             # 6 d-tiles
FT = F // 128             # 24 f-tiles
TT = TOK // 128           # 4 own-token tiles
HTT = HT // 128           # 8 halo-token tiles
NEG = -1e9
SCALE = 1.0 / 8.0


def _build(n_layers=L, debug=False):
    import concourse.bass as bass
    import concourse.mybir as mybir
    from concourse import bacc, tile
    import contextlib

    f32 = mybir.dt.float32
    bf16 = mybir.dt.bfloat16
    i32 = mybir.dt.int32
    AF = mybir.ActivationFunctionType
    OP = mybir.AluOpType

    nc = bacc.Bacc(None, num_devices=NCORE, target_bir_lowering=False)

    P = {}
    def par(name, shape, dt):
        P[name] = nc.declare_dram_parameter(name, list(shape), dt, isOutput=False)

    par('emb', [TOK, D], f32)
    par('embg', [128, DT], f32); par('embb', [128, DT], f32)
    for w in ['Wqt', 'Wkt', 'Wqgt', 'Wkgt', 'Wot']:
        par(w, [n_layers, DT, DT, 128, 128], bf16)        # [l, do, di, din128, dout128]
    par('Wvnat', [n_layers, DT, 128, H * 65], bf16)       # 65-stride per-head cols (+0 col)
    par('Wvgnat', [n_layers, DT, 128, 769], bf16)
    par('W1t', [n_layers, FT, DT, 128, 128], bf16)
    par('W2t', [n_layers, DT, FT, 128, 128], bf16)
    for b in ['bq', 'bk', 'bqg', 'bkg', 'bo', 'b2']:      # bq/bqg pre-scaled by 1/8 on host
        par(b, [n_layers, 128, DT], f32)
    par('bvrow', [n_layers, H * 65], bf16)                # bias cols + 1.0 at h*65+64
    par('bvgrow', [n_layers, 769], bf16)
    par('b1', [n_layers, 128, FT], f32)
    par('ln1g', [n_layers, 128, DT], f32); par('ln1b', [n_layers, 128, DT], f32)
    par('ln2g', [n_layers, 128, DT], f32); par('ln2b', [n_layers, 128, DT], f32)
    par('bandM', [128, HTT * TOK], bf16)
    par('amb', [128, TT], f32)
    par('selm', [128, DT], bf16)
    par('keepm', [128, DT], bf16)
    par('lidx', [128, DT], i32)
    par('ridx', [128, DT], i32)
    par('Wdt', [DT, DT, 128, 128], bf16)
    par('Wclst', [DT, 128, 2], bf16)
    par('Whh', [2, 128], bf16)
    par('Wopt', [128, 2], bf16)
    par('bd', [128, DT], f32); par('bcls', [2], f32)
    par('mg', [2], f32); par('mb', [2], f32)
    par('bh', [128], f32); par('bop', [2], f32)
    out_ext = nc.declare_dram_parameter('out', [2, 1], f32, isOutput=True)
    xout_ext = nc.declare_dram_parameter('xout', [128, DT * TOK], f32, isOutput=True)
    dbg = {}
    if debug:
        for nm, shape, dt in (
                ('d_xhT', [128, DT*HT], bf16), ('d_qT', [128, DT*TOK], bf16),
                ('d_kT', [128, DT*HT], bf16), ('d_vnat', [128, HTT*(H*65)], bf16),
                ('d_vgnat', [128, TT*769], bf16),
                ('d_x0', [128, 4*DT], bf16), ('d_v0row', [1, H*65], bf16),
                ('d_pT', [128, HTT*TOK], bf16), ('d_pgT', [128, TT*H], bf16),
                ('d_attn', [128, DT*TOK], bf16), ('d_ogs', [12, 65], f32),
                ('d_xmid', [128, DT*TOK], bf16), ('d_hT', [128, FT*TOK], bf16),
                ('d_pre1', [128, DT*TOK], f32)):
            dbg[nm] = nc.declare_dram_parameter(nm, shape, dt, isOutput=True)

    id_f32 = nc.inline_tensor(np.eye(128, dtype=np.float32), name='id_f32')
    id_bf = nc.inline_tensor(np.eye(128, dtype=np.float32).astype(bfloat16), name='id_bf')
    ones_row_np = np.ones((1, 128), dtype=np.float32)
    ones_col_np = np.ones((128, 1), dtype=np.float32)
    onesr_f = nc.inline_tensor(ones_row_np, name='onesr_f')
    onesr_b = nc.inline_tensor(ones_row_np.astype(bfloat16), name='onesr_b')
    onesc_f = nc.inline_tensor(ones_col_np, name='onesc_f')

    with tile.TileContext(nc) as tc:
        ctx = contextlib.ExitStack()
        with ctx:
            st = ctx.enter_context(tc.tile_pool(name='state', bufs=1))
            wsm = ctx.enter_context(tc.tile_pool(name='wsm', bufs=6))
            wnat = ctx.enter_context(tc.tile_pool(name='wnat', bufs=1))
            w2p = ctx.enter_context(tc.tile_pool(name='w2p', bufs=2))
            ptp = ctx.enter_context(tc.tile_pool(name='ptp', bufs=2))
            bp = ctx.enter_context(tc.tile_pool(name='bias', bufs=8))
            sp = ctx.enter_context(tc.tile_pool(name='small', bufs=1))
            hd = ctx.enter_context(tc.tile_pool(name='hd', bufs=2))
            sqp = ctx.enter_context(tc.tile_pool(name='sq', bufs=2))
            ps_proj = ctx.enter_context(tc.tile_pool(name='ps_proj', bufs=2, space='PSUM'))
            ps_st = ctx.enter_context(tc.tile_pool(name='ps_st', bufs=2, space='PSUM'))
            ps_pv = ctx.enter_context(tc.tile_pool(name='ps_pv', bufs=2, space='PSUM'))
            ps_misc = ctx.enter_context(tc.tile_pool(name='ps_misc', bufs=2, space='PSUM'))
            dram = ctx.enter_context(tc.tile_pool(name='dram', bufs=2, space='DRAM'))

            # ---------------- persistent state tiles ----------------
            xT = st.tile([128, DT * TOK], f32, tag='xT')
            xhT = st.tile([128, DT * HT], bf16, tag='xhT')
            qT = st.tile([128, DT * TOK], bf16, tag='qT')
            kT = st.tile([128, DT * HT], bf16, tag='kT')
            kgT = st.tile([128, DT * TOK], bf16, tag='kgT')
            vnat = st.tile([128, HTT * (H * 65)], bf16, tag='vnat')
            vgnat = st.tile([128, TT * 769], bf16, tag='vgnat')
            bandM_sb = st.tile([128, HTT * TOK], bf16, tag='bandM')
            attn_outT = st.tile([128, DT * TOK], bf16, tag='attn_outT')
            xmidb = st.tile([128, DT * TOK], bf16, tag='xmidb')
            hT = st.tile([128, FT * TOK], bf16, tag='hT')
            idsb = st.tile([128, 2 * DT], i32, tag='idsb')
            amb_sb = st.tile([128, TT], f32, tag='amb')
            selm_sb = st.tile([128, DT], bf16, tag='selm')
            keepm_sb = st.tile([128, DT], bf16, tag='keepm')
            idf_sb = st.tile([128, 128], f32, tag='idf')
            idb_sb = st.tile([128, 128], bf16, tag='idb')
            onesr_fs = st.tile([1, 128], f32, tag='onesr_f')
            onesr_bs = st.tile([1, 128], bf16, tag='onesr_b')
            onesc_fs = st.tile([128, 1], f32, tag='onesc_f')
            x0t = st.tile([128, DT], bf16, tag='x0t')
            k0t = st.tile([128, DT], bf16, tag='k0t')
            qgt = st.tile([128, DT], bf16, tag='qgt')
            v0row = st.tile([1, H * 65], bf16, tag='v0row')
            qgblk = st.tile([128, DT * H], bf16, tag='qgblk')
            pgT = st.tile([128, TT * H], bf16, tag='pgT')
            ogp = st.tile([12, 65], f32, tag='ogp')
            ogs = st.tile([12, 65], f32, tag='ogs')
            ogr = st.tile([12, 1], f32, tag='ogr')
            ogbf = st.tile([12, 64], bf16, tag='ogbf')
            ogT6 = st.tile([128, DT], bf16, tag='ogT6')
            tmp6 = st.tile([128, DT], bf16, tag='tmp6')
            eps_t = st.tile([128, 1], f32, tag='eps')
            nc.vector.memset(eps_t[:], 1e-5)

            dma = nc.sync.dma_start
            # one-time loads
            dma(out=idf_sb[:], in_=id_f32[:, :])
            dma(out=idb_sb[:], in_=id_bf[:, :])
            dma(out=onesr_fs[:], in_=onesr_f[:, :])
            dma(out=onesr_bs[:], in_=onesr_b[:, :])
            dma(out=onesc_fs[:], in_=onesc_f[:, :])
            dma(out=bandM_sb[:], in_=P['bandM'][:, :])
            dma(out=idsb[:, 0:DT], in_=P['lidx'][:, :])
            dma(out=idsb[:, DT:2*DT], in_=P['ridx'][:, :])
            dma(out=amb_sb[:], in_=P['amb'][:, :])
            dma(out=selm_sb[:], in_=P['selm'][:, :])
            dma(out=keepm_sb[:], in_=P['keepm'][:, :])

            # ---------------- embedding: LN then transpose (gamma/beta post-T) ----------------
            eg_t = bp.tile([128, DT], f32, tag='bqkv')
            dma(out=eg_t[:], in_=P['embg'][:, :])
            eb_t = bp.tile([128, DT], f32, tag='bqkv')
            dma(out=eb_t[:], in_=P['embb'][:, :])
            for t in range(TT):
                xe = sqp.tile([128, D], f32, tag='cen')
                dma(out=xe[:], in_=P['emb'][t*128:(t+1)*128, :])
                mu = sp.tile([128, 1], f32, tag='mu_e')
                nc.vector.reduce_sum(out=mu[:], in_=xe[:], axis=mybir.AxisListType.X)
                nc.scalar.mul(out=mu[:], in_=mu[:], mul=1.0/D)
                nc.vector.tensor_scalar_sub(out=xe[:], in0=xe[:], scalar1=mu[:, 0:1])
                sq = sqp.tile([128, D], f32, tag='cen')
                nc.scalar.square(out=sq[:], in_=xe[:])
                var = sp.tile([128, 1], f32, tag='mu_e')
                nc.vector.reduce_sum(out=var[:], in_=sq[:], axis=mybir.AxisListType.X)
                nc.scalar.activation(out=var[:], in_=var[:], func=AF.Sqrt,
                                     bias=eps_t[:, 0:1], scale=1.0/D)
                nc.vector.reciprocal(out=var[:], in_=var[:])
                nc.vector.tensor_scalar_mul(out=xe[:], in0=xe[:], scalar1=var[:, 0:1])
                for di in range(DT):
                    pt = ps_misc.tile([128, 128], f32, tag='m1')
                    nc.tensor.transpose(out=pt[:], in_=xe[:, di*128:(di+1)*128],
                                        identity=idf_sb[:])
                    nc.vector.tensor_scalar(
                        out=xT[:, di*TOK + t*128 : di*TOK + (t+1)*128], in0=pt[:],
                        scalar1=eg_t[:, di:di+1], scalar2=eb_t[:, di:di+1],
                        op0=OP.mult, op1=OP.add)
                    nc.scalar.copy(out=xhT[:, di*HT + BLK + t*128 : di*HT + BLK + (t+1)*128],
                                   in_=xT[:, di*TOK + t*128 : di*TOK + (t+1)*128])

            # ---------------- helpers ----------------
            def load_w6(wname, l, do, tag):
                wt = wsm.tile([128, DT * 128], bf16, tag=tag)
                dma(out=wt[:],
                    in_=P[wname][l, do, :, :, :].rearrange('di p c -> p di c'))
                return wt

            def load_bias_cols(pname, l, n, tag):
                bt = bp.tile([128, n], f32, tag=tag)
                dma(out=bt[:], in_=P[pname][l, :, :])
                return bt

            def layer_norm(l, src, dstf, dstb_fn, gname, bname):
                g_t = load_bias_cols(gname, l, DT, 'g_ln')
                b_t = load_bias_cols(bname, l, DT, 'b_ln')
                s1 = ps_misc.tile([1, TOK], f32, tag='m1')
                s2 = ps_misc.tile([1, TOK], f32, tag='m1')
                for di in range(DT):
                    nc.tensor.matmul(s1[:], lhsT=onesc_fs[:, :],
                                     rhs=src[:, di*TOK:(di+1)*TOK],
                                     start=(di == 0), stop=(di == DT-1))
                    sq = sqp.tile([128, TOK], f32, tag='sq')
                    nc.scalar.square(out=sq[:], in_=src[:, di*TOK:(di+1)*TOK])
                    nc.tensor.matmul(s2[:], lhsT=onesc_fs[:, :], rhs=sq[:],
                                     start=(di == 0), stop=(di == DT-1))
                mu = sp.tile([1, TOK], f32, tag='ln_mu')
                ex2 = sp.tile([1, TOK], f32, tag='ln_ex2')
                nc.scalar.mul(out=mu[:], in_=s1[:], mul=1.0/D)
                nc.scalar.mul(out=ex2[:], in_=s2[:], mul=1.0/D)
                var = sp.tile([1, TOK], f32, tag='ln_var')
                nc.vector.tensor_mul(out=var[:], in0=mu[:], in1=mu[:])
                nc.vector.tensor_sub(out=var[:], in0=ex2[:], in1=var[:])
                nc.scalar.activation(out=var[:], in_=var[:], func=AF.Sqrt,
                                     bias=eps_t[0:1, 0:1])
                nc.vector.reciprocal(out=var[:], in_=var[:])
                mub = ps_misc.tile([128, TOK], f32, tag='m1')
                rsb = ps_misc.tile([128, TOK], f32, tag='m1')
                nc.tensor.matmul(mub[:], lhsT=onesr_fs[:, :], rhs=mu[:], start=True, stop=True)
                nc.tensor.matmul(rsb[:], lhsT=onesr_fs[:, :], rhs=var[:], start=True, stop=True)
                rsbs = sqp.tile([128, TOK], f32, tag='sq')
                nc.scalar.copy(out=rsbs[:], in_=rsb[:])
                for di in range(DT):
                    cen = sqp.tile([128, TOK], f32, tag='cen')
                    nc.vector.tensor_sub(out=cen[:], in0=src[:, di*TOK:(di+1)*TOK], in1=mub[:])
                    nc.vector.tensor_mul(out=cen[:], in0=cen[:], in1=rsbs[:])
                    nc.vector.tensor_scalar(
                        out=dstf[:, di*TOK:(di+1)*TOK], in0=cen[:],
                        scalar1=g_t[:, di:di+1], scalar2=b_t[:, di:di+1],
                        op0=OP.mult, op1=OP.add)
                    nc.scalar.copy(out=dstb_fn(di), in_=dstf[:, di*TOK:(di+1)*TOK])

            # ---------------- layers ----------------
            for l in range(n_layers):
                ag_in = dram.tile([D, TOK], bf16, tag='ag_in')
                ag_out = dram.tile([NCORE * D, TOK], bf16, tag='ag_out')
                for di in range(DT):
                    dma(out=ag_in[di*128:(di+1)*128, :],
                        in_=xhT[:, di*HT + BLK : di*HT + BLK + TOK])
                nc.gpsimd.collective_compute(
                    'AllGather', OP.bypass,
                    replica_groups=[list(range(NCORE))],
                    ins=[ag_in[:].opt()], outs=[ag_out[:].opt()])
                for di in range(DT):
                    nc.gpsimd.indirect_dma_start(
                        out=xhT[:, di*HT : di*HT + BLK], out_offset=None,
                        in_=ag_out[:],
                        in_offset=bass.IndirectOffsetOnAxis(ap=idsb[:, di:di+1], axis=0),
                        element_offset=TOK - BLK)
                    nc.gpsimd.indirect_dma_start(
                        out=xhT[:, di*HT + BLK + TOK : (di+1)*HT], out_offset=None,
                        in_=ag_out[:],
                        in_offset=bass.IndirectOffsetOnAxis(ap=idsb[:, DT+di:DT+di+1], axis=0),
                        element_offset=0)
                for di in range(DT):
                    dma(out=x0t[:, di:di+1], in_=ag_out[di*128:(di+1)*128, 0:1])

                if debug and l == 0:
                    dma(out=dbg['d_xhT'][:, :], in_=xhT[:])
                bq_t = load_bias_cols('bq', l, DT, 'bqkv')
                bk_t = load_bias_cols('bk', l, DT, 'bqkv')
                bqg_t = load_bias_cols('bqg', l, DT, 'bqkv')
                bkg_t = load_bias_cols('bkg', l, DT, 'bqkv')
                bo_t = load_bias_cols('bo', l, DT, 'bqkv')
                b2_t = load_bias_cols('b2', l, DT, 'bqkv')
                b1_t = load_bias_cols('b1', l, FT, 'b1')
                bvrow_t = bp.tile([1, H * 65], bf16, tag='bvrow')
                dma(out=bvrow_t[:], in_=P['bvrow'][l, :].rearrange('(o d) -> o d', o=1))
                bvgrow_t = bp.tile([1, 769], bf16, tag='bvrow')
                dma(out=bvgrow_t[:], in_=P['bvgrow'][l, :].rearrange('(o d) -> o d', o=1))

                # ---- own-token projections first (overlap with AllGather) ----
                def proj_T(wname, dst, bias_t, span, off, scl):
                    for do in range(DT):
                        wt = load_w6(wname, l, do, 'w_sm')
                        for n0 in range(0, span, 512):
                            pp = ps_proj.tile([128, 512], f32, tag='pp')
                            for di in range(DT):
                                nc.tensor.matmul(
                                    pp[:],
                                    lhsT=wt[:, di*128:(di+1)*128],
                                    rhs=xhT[:, di*HT + off + n0 : di*HT + off + n0 + 512],
                                    start=(di == 0), stop=(di == DT-1))
                            nc.scalar.activation(
                                out=dst[:, do*span + n0 : do*span + n0 + 512], in_=pp[:],
                                func=AF.Identity, bias=bias_t[:, do:do+1], scale=scl)
                proj_T('Wqt', qT, bq_t, TOK, BLK, SCALE)
                proj_T('Wkgt', kgT, bkg_t, TOK, BLK, 1.0)
                # ---- natural-layout v projections ----
                wv_t = [wnat.tile([128, H * 65], bf16, tag='wv' + str(di), name=f'wv_{l}_{di}') for di in range(DT)]
                for di in range(DT):
                    dma(out=wv_t[di][:], in_=P['Wvnat'][l, di, :, :])
                def vnat_tiles(trange):
                    for t in trange:
                        for n0 in range(0, H * 65, 390):
                            n1 = n0 + 390
                            pp = ps_proj.tile([128, 512], f32, tag='pp')
                            for di in range(DT):
                                nc.tensor.matmul(
                                    pp[:, :390],
                                    lhsT=xhT[:, di*HT + t*128 : di*HT + (t+1)*128],
                                    rhs=wv_t[di][:, n0:n1], start=(di == 0), stop=False)
                            nc.tensor.matmul(pp[:, :390], lhsT=onesr_bs[:, :],
                                             rhs=bvrow_t[:, n0:n1], start=False, stop=True)
                            nc.scalar.copy(out=vnat[:, t*(H*65) + n0 : t*(H*65) + n1],
                                           in_=pp[:, :390])
                vnat_tiles([2, 3, 4, 5])
                # halo-dependent work after own-token work:
                proj_T('Wkt', kT, bk_t, HT, 0, 1.0)
                vnat_tiles([0, 1, 6, 7])
                # x0 trio: k0 / qg columns from gathered token 0
                for (wname, x0dst, bias_t2, scl2) in (('Wkt', k0t, bk_t, 1.0),
                                                      ('Wqgt', qgt, bqg_t, SCALE)):
                    for do in range(DT):
                        wt = load_w6(wname, l, do, 'w_sm')
                        px = ps_misc.tile([128, 1], f32, tag='m1')
                        for di in range(DT):
                            nc.tensor.matmul(px[:], lhsT=wt[:, di*128:(di+1)*128],
                                             rhs=x0t[:, di:di+1],
                                             start=(di == 0), stop=(di == DT-1))
                        nc.scalar.activation(out=x0dst[:, do:do+1], in_=px[:],
                                             func=AF.Identity, bias=bias_t2[:, do:do+1],
                                             scale=scl2)
                # v0 row directly from Wvnat                # v0 row directly from Wvnat (token-0 column as lhsT)
                for n0 in range(0, H * 65, 390):
                    n1 = n0 + 390
                    pv0 = ps_misc.tile([1, 512], f32, tag='m1')
                    for di in range(DT):
                        nc.tensor.matmul(pv0[:, :390], lhsT=x0t[:, di:di+1],
                                         rhs=wv_t[di][:, n0:n1], start=(di == 0), stop=False)
                    nc.tensor.matmul(pv0[:, :390], lhsT=onesr_bs[:, 0:1],
                                     rhs=bvrow_t[:, n0:n1], start=False, stop=True)
                    nc.scalar.copy(out=v0row[:, n0:n1], in_=pv0[:, :390])

                wvg_t = [wnat.tile([128, 769], bf16, tag='wv' + str(di), name=f'wvg_{l}_{di}') for di in range(DT)]
                for di in range(DT):
                    dma(out=wvg_t[di][:], in_=P['Wvgnat'][l, di, :, :])
                for t in range(TT):
                    for n0 in range(0, 769, 512):
                        n1 = min(n0 + 512, 769)
                        pp = ps_proj.tile([128, 512], f32, tag='pp')
                        for di in range(DT):
                            nc.tensor.matmul(
                                pp[:, :n1-n0],
                                lhsT=xhT[:, di*HT + BLK + t*128 : di*HT + BLK + (t+1)*128],
                                rhs=wvg_t[di][:, n0:n1], start=(di == 0), stop=False)
                        nc.tensor.matmul(pp[:, :n1-n0], lhsT=onesr_bs[:, :],
                                         rhs=bvgrow_t[:, n0:n1], start=False, stop=True)
                        nc.scalar.copy(out=vgnat[:, t*769 + n0 : t*769 + n1],
                                       in_=pp[:, :n1-n0])

                if debug and l == 0:
                    dma(out=dbg['d_vnat'][:, :], in_=vnat[:])
                    dma(out=dbg['d_vgnat'][:, :], in_=vgnat[:])
                    dma(out=dbg['d_v0row'][:, :], in_=v0row[:])
                # ---- global row ----
                nc.vector.memset(qgblk[:], 0.0)
                for h in range(H):
                    nc.scalar.copy(
                        out=qgblk[(h % 2)*64:(h % 2)*64+64, (h//2)*H + h : (h//2)*H + h + 1],
                        in_=qgt[(h % 2)*64:(h % 2)*64+64, h//2:h//2+1])
                for t in range(TT):
                    sgt = ps_misc.tile([128, H], f32, tag='m1')
                    for di in range(DT):
                        nc.tensor.matmul(sgt[:],
                                         lhsT=kgT[:, di*TOK + t*128 : di*TOK + (t+1)*128],
                                         rhs=qgblk[:, di*H:(di+1)*H],
                                         start=(di == 0), stop=(di == DT-1))
                    nc.scalar.activation(out=pgT[:, t*H:(t+1)*H], in_=sgt[:], func=AF.Exp,
                                         bias=amb_sb[:, t:t+1], scale=1.0)
                oga = ps_pv.tile([12, 512], f32, tag='pv')
                ogb = ps_pv.tile([12, 257], f32, tag='pv')
                for t in range(TT):
                    nc.tensor.matmul(oga[:], lhsT=pgT[:, t*H:(t+1)*H],
                                     rhs=vgnat[:, t*769 : t*769 + 512],
                                     start=(t == 0), stop=(t == TT-1))
                for t in range(TT):
                    nc.tensor.matmul(ogb[:], lhsT=pgT[:, t*H:(t+1)*H],
                                     rhs=vgnat[:, t*769 + 512 : (t+1)*769],
                                     start=(t == 0), stop=(t == TT-1))
                ogsb_a = sqp.tile([12, 512], f32, tag='sq')
                ogsb_b = sqp.tile([12, 257], f32, tag='sq')
                nc.scalar.copy(out=ogsb_a[:], in_=oga[:])
                nc.scalar.copy(out=ogsb_b[:], in_=ogb[:])
                ar_in = dram.tile([12, 65], f32, tag='ar_in')
                ar_out = dram.tile([12, 65], f32, tag='ar_out')
                for h in range(H):
                    if h < 8:
                        dma(out=ar_in[h:h+1, 0:64], in_=ogsb_a[h:h+1, h*64:(h+1)*64])
                    else:
                        dma(out=ar_in[h:h+1, 0:64],
                            in_=ogsb_b[h:h+1, h*64-512:(h+1)*64-512])
                dma(out=ar_in[:, 64:65], in_=ogsb_b[:, 256:257])
                nc.gpsimd.collective_compute(
                    'AllReduce', OP.add, replica_groups=[list(range(NCORE))],
                    ins=[ar_in[:].opt()], outs=[ar_out[:].opt()])
                dma(out=ogs[:], in_=ar_out[:])
                # ---- band attention per head ----
                for h in range(H):
                    po = (h % 2) * 64
                    dt_ = h // 2
                    pT = ptp.tile([128, HTT * TOK], bf16, tag='pT')
                    sg = ps_misc.tile([1, TOK], f32, tag='m1')
                    nc.tensor.matmul(sg[:], lhsT=k0t[po:po+64, dt_:dt_+1],
                                     rhs=qT[po:po+64, dt_*TOK:(dt_+1)*TOK],
                                     start=True, stop=True)
                    pglob = hd.tile([1, TOK], bf16, tag='pglob')
                    nc.scalar.activation(out=pglob[:], in_=sg[:], func=AF.Exp)
                    for c in range(HTT):
                        stp = ps_st.tile([128, TOK], f32, tag='st')
                        nc.tensor.matmul(
                            stp[:],
                            lhsT=kT[po:po+64, dt_*HT + c*128 : dt_*HT + (c+1)*128],
                            rhs=qT[po:po+64, dt_*TOK:(dt_+1)*TOK],
                            start=True, stop=True)
                        nc.scalar.activation(out=pT[:, c*TOK:(c+1)*TOK], in_=stp[:],
                                             func=AF.Exp)
                        nc.vector.tensor_mul(out=pT[:, c*TOK:(c+1)*TOK],
                                             in0=pT[:, c*TOK:(c+1)*TOK],
                                             in1=bandM_sb[:, c*TOK:(c+1)*TOK])
                    pvp = ps_pv.tile([65, TOK], f32, tag='pv')
                    for c in range(HTT):
                        nc.tensor.matmul(
                            pvp[:],
                            lhsT=vnat[:, c*(H*65) + h*65 : c*(H*65) + h*65 + 65],
                            rhs=pT[:, c*TOK:(c+1)*TOK], start=(c == 0), stop=False)
                    nc.tensor.matmul(pvp[:], lhsT=v0row[:, h*65:h*65+65], rhs=pglob[:],
                                     start=False, stop=True)
                    if debug and l == 0 and h == 0:
                        dma(out=dbg['d_pT'][:, :], in_=pT[:])
                    rden = hd.tile([1, TOK], f32, tag='rden')
                    nc.vector.reciprocal(out=rden[:], in_=pvp[64:65, :])
                    rb = ps_misc.tile([64, TOK], f32, tag='m1')
                    nc.tensor.matmul(rb[:], lhsT=onesr_fs[:, 0:64], rhs=rden[:],
                                     start=True, stop=True)
                    rbs = sqp.tile([64, TOK], f32, tag='sq')
                    nc.scalar.copy(out=rbs[:], in_=rb[:])
                    nc.vector.tensor_mul(out=attn_outT[po:po+64, dt_*TOK:(dt_+1)*TOK],
                                         in0=pvp[0:64, :], in1=rbs[:])

                nc.vector.reciprocal(out=ogr[:], in_=ogs[:, 64:65])
                nc.vector.tensor_scalar_mul(out=ogbf[:], in0=ogs[:, 0:64],
                                            scalar1=ogr[:, 0:1])
                ogt_ps = ps_misc.tile([64, 12], bf16, tag='m1')
                nc.tensor.transpose(out=ogt_ps[:], in_=ogbf[:], identity=idb_sb[0:12, 0:12])
                ogT = sp.tile([64, 12], bf16, tag='ogT')
                nc.scalar.copy(out=ogT[:], in_=ogt_ps[:])
                for h in range(H):
                    dma(out=ogT6[(h % 2)*64:(h % 2)*64+64, h//2:h//2+1],
                        in_=ogT[:, h:h+1])
                nc.vector.tensor_mul(out=tmp6[:], in0=ogT6[:], in1=selm_sb[:])
                for t6 in range(DT):
                    c0 = attn_outT[:, t6*TOK : t6*TOK + 1]
                    nc.vector.tensor_mul(out=c0, in0=c0, in1=keepm_sb[:, t6:t6+1])
                    nc.vector.tensor_add(out=c0, in0=c0, in1=tmp6[:, t6:t6+1])

                if debug and l == 0:
                    dma(out=dbg['d_pgT'][:, :], in_=pgT[:])
                    dma(out=dbg['d_ogs'][:, :], in_=ogs[:])
                    dma(out=dbg['d_attn'][:, :], in_=attn_outT[:])
                # ---- output projection + residual + LN1 ----
                for do in range(DT):
                    wt = load_w6('Wot', l, do, 'w_sm')
                    pp = ps_proj.tile([128, 512], f32, tag='pp')
                    for di in range(DT):
                        nc.tensor.matmul(pp[:], lhsT=wt[:, di*128:(di+1)*128],
                                         rhs=attn_outT[:, di*TOK:(di+1)*TOK],
                                         start=(di == 0), stop=(di == DT-1))
                    nc.vector.scalar_tensor_tensor(
                        out=xT[:, do*TOK:(do+1)*TOK], in0=pp[:], scalar=bo_t[:, do:do+1],
                        in1=xT[:, do*TOK:(do+1)*TOK], op0=OP.add, op1=OP.add)

                if debug and l == 0:
                    dma(out=dbg['d_pre1'][:, :], in_=xT[:])
                layer_norm(l, xT, xT, lambda di: xmidb[:, di*TOK:(di+1)*TOK],
                           'ln1g', 'ln1b')
                if debug and l == 0:
                    dma(out=dbg['d_xmid'][:, :], in_=xmidb[:])

                # ---- FFN ----
                for do in range(FT):
                    wt = load_w6('W1t', l, do, 'w_sm')
                    pp = ps_proj.tile([128, 512], f32, tag='pp')
                    for di in range(DT):
                        nc.tensor.matmul(pp[:], lhsT=wt[:, di*128:(di+1)*128],
                                         rhs=xmidb[:, di*TOK:(di+1)*TOK],
                                         start=(di == 0), stop=(di == DT-1))
                    nc.scalar.activation(out=hT[:, do*TOK:(do+1)*TOK], in_=pp[:],
                                         func=AF.Gelu, bias=b1_t[:, do:do+1])
                if debug and l == 0:
                    dma(out=dbg['d_hT'][:, :], in_=hT[:])
                for do in range(DT):
                    wt = w2p.tile([128, FT * 128], bf16, tag='w2')
                    dma(out=wt[:],
                        in_=P['W2t'][l, do, :, :, :].rearrange('di p c -> p di c'))
                    pp = ps_proj.tile([128, 512], f32, tag='pp')
                    for di in range(FT):
                        nc.tensor.matmul(pp[:], lhsT=wt[:, di*128:(di+1)*128],
                                         rhs=hT[:, di*TOK:(di+1)*TOK],
                                         start=(di == 0), stop=(di == FT-1))
                    nc.vector.scalar_tensor_tensor(
                        out=xT[:, do*TOK:(do+1)*TOK], in0=pp[:], scalar=b2_t[:, do:do+1],
                        in1=xT[:, do*TOK:(do+1)*TOK], op0=OP.add, op1=OP.add)

                layer_norm(l, xT, xT,
                           lambda di: xhT[:, di*HT + BLK : di*HT + BLK + TOK],
                           'ln2g', 'ln2b')

            # ---------------- head (token 0; real on core 0) ----------------
            x0f = sp.tile([128, DT], bf16, tag='x0f')
            for di in range(DT):
                nc.scalar.copy(out=x0f[:, di:di+1], in_=xT[:, di*TOK:di*TOK+1])
            bd_t = bp.tile([128, DT], f32, tag='bqkv')
            dma(out=bd_t[:], in_=P['bd'][:, :])
            clsT = sp.tile([128, DT], bf16, tag='clsT')
            for do in range(DT):
                wt = wsm.tile([128, DT * 128], bf16, tag='w_sm')
                dma(out=wt[:],
                    in_=P['Wdt'][do, :, :, :].rearrange('di p c -> p di c'))
                pp = ps_misc.tile([128, 1], f32, tag='m1')
                for di in range(DT):
                    nc.tensor.matmul(pp[:], lhsT=wt[:, di*128:(di+1)*128],
                                     rhs=x0f[:, di:di+1], start=(di == 0), stop=(di == DT-1))
                nc.scalar.activation(out=clsT[:, do:do+1], in_=pp[:], func=AF.Tanh,
                                     bias=bd_t[:, do:do+1])
            wcls = sp.tile([128, DT*2], bf16, tag='wcls')
            dma(out=wcls[:], in_=P['Wclst'][:, :, :].rearrange('t p c -> p t c'))
            lgp = ps_misc.tile([2, 1], f32, tag='m1')
            for di in range(DT):
                nc.tensor.matmul(lgp[:], lhsT=wcls[:, di*2:(di+1)*2], rhs=clsT[:, di:di+1],
                                 start=(di == 0), stop=(di == DT-1))
            bcls_t = sp.tile([2, 1], f32, tag='bcls')
            dma(out=bcls_t[:], in_=P['bcls'].ap().rearrange('(c o) -> c o', o=1))
            lgT = sp.tile([2, 1], f32, tag='lgT')
            nc.vector.tensor_add(out=lgT[:], in0=lgp[:], in1=bcls_t[:])
            lg_ps = ps_misc.tile([1, 2], f32, tag='m1')
            nc.tensor.matmul(lg_ps[:], lhsT=lgT[:], rhs=idf_sb[0:2, 0:2], start=True, stop=True)
            lg = sp.tile([1, 2], f32, tag='lg')
            nc.scalar.copy(out=lg[:], in_=lg_ps[:])
            muh = sp.tile([1, 1], f32, tag='h_mu')
            nc.vector.reduce_sum(out=muh[:], in_=lg[:], axis=mybir.AxisListType.X)
            nc.scalar.mul(out=muh[:], in_=muh[:], mul=0.5)
            nc.vector.tensor_scalar_sub(out=lg[:], in0=lg[:], scalar1=muh[:, 0:1])
            sqh = sp.tile([1, 2], f32, tag='h_sq')
            nc.scalar.square(out=sqh[:], in_=lg[:])
            varh = sp.tile([1, 1], f32, tag='h_var')
            nc.vector.reduce_sum(out=varh[:], in_=sqh[:], axis=mybir.AxisListType.X)
            nc.scalar.activation(out=varh[:], in_=varh[:], func=AF.Sqrt,
                                 bias=eps_t[0:1, 0:1], scale=0.5)
            nc.vector.reciprocal(out=varh[:], in_=varh[:])
            nc.vector.tensor_scalar_mul(out=lg[:], in0=lg[:], scalar1=varh[:, 0:1])
            mg_t = sp.tile([1, 2], f32, tag='mg')
            dma(out=mg_t[:], in_=P['mg'].ap().rearrange('(o c) -> o c', o=1))
            mb_t = sp.tile([1, 2], f32, tag='mbt')
            dma(out=mb_t[:], in_=P['mb'].ap().rearrange('(o c) -> o c', o=1))
            nc.vector.tensor_mul(out=lg[:], in0=lg[:], in1=mg_t[:])
            nc.vector.tensor_add(out=lg[:], in0=lg[:], in1=mb_t[:])
            one1 = sp.tile([1, 1], f32, tag='one1')
            nc.vector.memset(one1[:], 1.0)
            yt_ps = ps_misc.tile([2, 1], f32, tag='m1')
            nc.tensor.matmul(yt_ps[:], lhsT=lg[:], rhs=one1[:], start=True, stop=True)
            yT = sp.tile([2, 1], bf16, tag='yT')
            nc.scalar.copy(out=yT[:], in_=yt_ps[:])
            whh = sp.tile([2, 128], bf16, tag='whh')
            dma(out=whh[:], in_=P['Whh'][:, :])
            h1p = ps_misc.tile([128, 1], f32, tag='m1')
            nc.tensor.matmul(h1p[:], lhsT=whh[:], rhs=yT[:], start=True, stop=True)
            bh_t = sp.tile([128, 1], f32, tag='bh')
            dma(out=bh_t[:], in_=P['bh'].ap().rearrange('(p o) -> p o', o=1))
            h1 = sp.tile([128, 1], bf16, tag='h1')
            nc.scalar.activation(out=h1[:], in_=h1p[:], func=AF.Relu, bias=bh_t[:, 0:1])
            wop = sp.tile([128, 2], bf16, tag='wop')
            dma(out=wop[:], in_=P['Wopt'][:, :])
            o2p = ps_misc.tile([2, 1], f32, tag='m1')
            nc.tensor.matmul(o2p[:], lhsT=wop[:], rhs=h1[:], start=True, stop=True)
            bop_t = sp.tile([2, 1], f32, tag='bop')
            dma(out=bop_t[:], in_=P['bop'].ap().rearrange('(c o) -> c o', o=1))
            o2 = sp.tile([2, 1], f32, tag='o2')
            nc.vector.tensor_add(out=o2[:], in0=o2p[:], in1=bop_t[:])
            dma(out=out_ext[:, :], in_=o2[:])
            dma(out=xout_ext[:, :], in_=xT[:])

    nc.finalize()
    return nc


def _host_prep(inputs, n_layers=L):
    f32 = np.float32
    ids = np.asarray(inputs['x_ids']).reshape(-1).astype(np.int64)
    amask = np.asarray(inputs['attention_mask']).reshape(-1)
    gmask = np.asarray(inputs['global_attention_mask']).reshape(-1)
    W = {k: np.asarray(v, dtype=f32) for k, v in inputs.items()
         if k not in ('x_ids', 'attention_mask', 'global_attention_mask')}

    c = np.arange(3*BLK); r = np.arange(BLK)
    band_ok = np.abs(c[None, :] - BLK - r[:, None]) <= BLK
    key_idx = np.arange(NB)[:, None]*BLK - BLK + c[None, :]
    in_rng = (key_idx >= 0) & (key_idx < S)
    safe = np.clip(key_idx, 0, S-1)
    key_ok = in_rng & (amask[safe] > 0) & (gmask[safe] <= 0)
    band_mask = band_ok[None, :, :] & key_ok[:, None, :]   # [NB, BLK(q), 3BLK(k)]

    emb_full = (W['word_emb'][ids] + W['pos_emb'][np.arange(S)+2]
                + W['type_emb']).astype(f32)

    def tiles_lhsT(w):   # [din, dout] -> [do, di, 128, 128]
        din, dout = w.shape
        t = w.reshape(din//128, 128, dout//128, 128)
        return np.ascontiguousarray(t.transpose(2, 0, 1, 3)).astype(bfloat16)

    def stack_l(w):
        if n_layers == 0:
            return np.zeros((0,), np.float32).astype(bfloat16)
        return np.stack([tiles_lhsT(w[l]) for l in range(n_layers)])

    cm = {}
    cm['Wqt'] = stack_l(W['Wq']); cm['Wkt'] = stack_l(W['Wk'])
    cm['Wqgt'] = stack_l(W['Wqg']); cm['Wkgt'] = stack_l(W['Wkg'])
    cm['Wot'] = stack_l(W['Wo'])
    wv_ext = np.zeros((n_layers, DT, 128, H, 65), np.float32)
    wv_ext[:, :, :, :, 0:64] = W['Wv'][:n_layers].reshape(n_layers, DT, 128, H, 64)
    cm['Wvnat'] = np.ascontiguousarray(
        wv_ext.reshape(n_layers, DT, 128, H * 65)).astype(bfloat16)
    wvg_ext = np.zeros((n_layers, DT, 128, 769), np.float32)
    wvg_ext[:, :, :, 0:768] = W['Wvg'][:n_layers].reshape(n_layers, DT, 128, D)
    cm['Wvgnat'] = np.ascontiguousarray(wvg_ext).astype(bfloat16)
    cm['W1t'] = stack_l(W['W1']); cm['W2t'] = stack_l(W['W2'])
    def bcols(v, n):   # [L, n*128] -> [L, 128, n]
        return np.ascontiguousarray(
            v.reshape(n_layers, n, 128).transpose(0, 2, 1)).astype(f32)
    cm['bq'] = bcols(W['bq'][:n_layers] * SCALE, DT)
    cm['bqg'] = bcols(W['bqg'][:n_layers] * SCALE, DT)
    for b in ['bk', 'bkg', 'bo', 'b2']:
        cm[b] = bcols(W[b][:n_layers], DT)
    cm['b1'] = bcols(W['b1'][:n_layers], FT)
    bv_ext = np.zeros((n_layers, H, 65), np.float32)
    bv_ext[:, :, 0:64] = W['bv'][:n_layers].reshape(n_layers, H, 64)
    bv_ext[:, :, 64] = 1.0
    cm['bvrow'] = np.ascontiguousarray(bv_ext.reshape(n_layers, H*65)).astype(bfloat16)
    bvg_ext = np.zeros((n_layers, 769), np.float32)
    bvg_ext[:, 0:768] = W['bvg'][:n_layers]
    bvg_ext[:, 768] = 1.0
    cm['bvgrow'] = np.ascontiguousarray(bvg_ext).astype(bfloat16)
    cm['ln1g'] = bcols(W['ln1_g'][:n_layers], DT)
    cm['ln1b'] = bcols(W['ln1_b'][:n_layers], DT)
    cm['ln2g'] = bcols(W['ln2_g'][:n_layers], DT)
    cm['ln2b'] = bcols(W['ln2_b'][:n_layers], DT)
    cm['embg'] = np.ascontiguousarray(
        W['emb_ln_g'].reshape(DT, 128).T).astype(f32)
    cm['embb'] = np.ascontiguousarray(
        W['emb_ln_b'].reshape(DT, 128).T).astype(f32)
    cm['Wdt'] = tiles_lhsT(W['Wd'])
    cm['Wclst'] = np.ascontiguousarray(W['Wcls'].reshape(DT, 128, 2)).astype(bfloat16)
    cm['Whh'] = W['Wh'].astype(bfloat16)
    cm['Wopt'] = W['Wop'].astype(bfloat16)
    cm['bd'] = np.ascontiguousarray(W['bd'].reshape(DT, 128).T).astype(f32)
    cm['bcls'] = W['bcls'].astype(f32)
    sm = np.exp(W['mix_w'] - W['mix_w'].max()); sm = sm / sm.sum()
    cm['mg'] = (W['mln_g'] * sm.sum()).astype(f32)
    cm['mb'] = W['mln_b'].astype(f32)
    cm['bh'] = W['bh'].astype(f32)
    cm['bop'] = W['bop'].astype(f32)

    in_maps = []
    for core in range(NCORE):
        m = dict(cm)
        b0, b1_ = 2*core, 2*core + 1
        M = np.zeros((HT, TOK), f32)
        M[0:3*BLK, 0:BLK] = band_mask[b0].T
        M[BLK:BLK+3*BLK, BLK:2*BLK] = band_mask[b1_].T
        m['bandM'] = np.ascontiguousarray(
            M.reshape(HTT, 128, TOK).transpose(1, 0, 2).reshape(128, HTT*TOK)).astype(bfloat16)
        amb = np.where(amask[core*TOK:(core+1)*TOK] > 0, 0.0, NEG).astype(f32)
        m['amb'] = np.ascontiguousarray(amb.reshape(TT, 128).T).astype(f32)
        sel = np.full((128, DT), 1.0 if core == 0 else 0.0, f32)
        m['selm'] = sel.astype(bfloat16)
        m['keepm'] = (1.0 - sel).astype(bfloat16)
        left = max(core - 1, 0); right = min(core + 1, NCORE - 1)
        m['lidx'] = np.ascontiguousarray(
            (left*D + np.arange(D)).reshape(DT, 128).T).astype(np.int32)
        m['ridx'] = np.ascontiguousarray(
            (right*D + np.arange(D)).reshape(DT, 128).T).astype(np.int32)
        m['emb'] = emb_full[core*TOK:(core+1)*TOK]
        in_maps.append(m)
    return in_maps


_NC_CACHE = {}

def kernel(_n_layers=L, _want_hidden=False, _spmd_kwargs=None, _debug=False, **inputs):
    from concourse.bass_utils import run_bass_kernel_spmd
    n_layers = _n_layers
    key = (n_layers, _debug)
    if key not in _NC_CACHE:
        _NC_CACHE[key] = _build(n_layers, debug=_debug)
    nc = _NC_CACHE[key]
    in_maps = _host_prep(inputs, n_layers)
    res = run_bass_kernel_spmd(nc, in_maps, core_ids=list(range(NCORE)),
                               **(_spmd_kwargs or {}))
    out = np.asarray(res.results[0]['out']).reshape(1, 2)
    if _want_hidden:
        hid = np.concatenate(
            [np.asarray(res.results[c]['xout']).reshape(128, DT, TOK)
             .transpose(2, 1, 0).reshape(TOK, D) for c in range(NCORE)], axis=0)
        return out.astype(np.float32), hid, res
    return out.astype(np.float32)



# revision 21
# speedup vs baseline: 1.2346x; 1.2346x over previous
"""Trainium2 Bass kernel for nn_ACMnnAttnEmb (Longformer-style, S=4096, 12 layers).

Sequence-parallel over 8 NeuronCores: 512 tokens (2 attention blocks) per core.
Per layer: one bf16 AllGather of the residual stream (halo + global token) and
one tiny f32 AllReduce for the global-attention row. All matmuls bf16 with f32
PSUM accumulation; LayerNorm/softmax in f32. Softmax is computed without
max-subtraction (scores empirically bounded |s| < 3) with denominators folded
into the PV matmul via a ones-column in V.

v2: band attention restricted to live query ranges per key chunk (62.5% of the
dense work), softmax denominators via fast-reciprocal + gpsimd partition
broadcast, token-0 (global) output merged as a rank-1 correction in the Wo
projection so the AllReduce is off the critical path, x0 projections folded
into the Wk weight pass, partition-contiguous weight layouts.
"""
import numpy as np
from ml_dtypes import bfloat16

NCORE = 8
S = 4096; D = 768; H = 12; DH = 64; F = 3072; L = 12
BLK = 256; NB = S // BLK
TOK = S // NCORE          # 512 tokens per core
HT = TOK + 2 * BLK        # 1024-token halo window
DT = D // 128             # 6 d-tiles
FT = F // 128             # 24 f-tiles
TT = TOK // 128           # 4 own-token tiles
HTT = HT // 128           # 8 halo-token tiles
NEG = -1e9
SCALE = 1.0 / 8.0

# live query range per key chunk c (keys [128c,128c+128) of the halo):
# q in [max(0,128(c-4)), min(512,128(c+1)))
LIVE = [128, 256, 384, 512, 512, 384, 256, 128]
QLO = [0, 0, 0, 0, 0, 128, 256, 384]
OFF = [0, 128, 384, 768, 1280, 1792, 2176, 2432]
PTW = 2560
CORDER = [3, 4, 0, 1, 2, 5, 6, 7]   # full-width chunks first (PSUM init)


def _build(n_layers=L, debug=False):
    import concourse.bass as bass
    import concourse.mybir as mybir
    from concourse import bacc, tile
    import contextlib

    f32 = mybir.dt.float32
    bf16 = mybir.dt.bfloat16
    i32 = mybir.dt.int32
    AF = mybir.ActivationFunctionType
    OP = mybir.AluOpType

    nc = bacc.Bacc(None, num_devices=NCORE, target_bir_lowering=False)

    P = {}
    def par(name, shape, dt):
        P[name] = nc.declare_dram_parameter(name, list(shape), dt, isOutput=False)

    par('emb', [TOK, D], f32)
    par('embg', [128, DT], f32); par('embb', [128, DT], f32)
    # weights: partition-contiguous lhsT tiles [*, 128p, DT*128]
    for w in ['Wqt', 'Wkt', 'Wqgt', 'Wkgt', 'Wot']:
        par(w, [n_layers, DT, 128, DT * 128], bf16)
    par('Wvnat', [n_layers, DT, 128, H * 65], bf16)       # 65-stride per-head cols (+0 col)
    par('Wvgnat', [n_layers, DT, 128, 769], bf16)
    par('W1t', [n_layers, FT, 128, DT * 128], bf16)
    par('W2t', [n_layers, DT, 128, FT * 128], bf16)
    for b in ['bq', 'bk', 'bqg', 'bkg', 'bo', 'b2']:      # bq/bqg pre-scaled by 1/8 on host
        par(b, [n_layers, 128, DT], f32)
    par('bvrow', [n_layers, H * 65], bf16)                # bias cols + 1.0 at h*65+64
    par('bvgrow', [n_layers, 769], bf16)
    par('b1', [n_layers, 128, FT], f32)
    par('ln1g', [n_layers, 128, DT], f32); par('ln1b', [n_layers, 128, DT], f32)
    par('ln2g', [n_layers, 128, DT], f32); par('ln2b', [n_layers, 128, DT], f32)
    par('bandM', [128, PTW], bf16)
    par('amb', [128, TT], f32)
    par('selm', [128, DT], bf16)
    par('lidx', [128, DT], i32)
    par('ridx', [128, DT], i32)
    par('Wdt', [DT, 128, DT * 128], bf16)
    par('Wclst', [DT, 128, 2], bf16)
    par('Whh', [2, 128], bf16)
    par('Wopt', [128, 2], bf16)
    par('bd', [128, DT], f32); par('bcls', [2], f32)
    par('mg', [2], f32); par('mb', [2], f32)
    par('bh', [128], f32); par('bop', [2], f32)
    out_ext = nc.declare_dram_parameter('out', [2, 1], f32, isOutput=True)
    xout_ext = nc.declare_dram_parameter('xout', [128, DT * TOK], f32, isOutput=True)
    dbg = {}
    if debug:
        for nm, shape, dt in (
                ('d_qT', [128, DT*TOK], bf16), ('d_kT', [128, DT*HT], bf16),
                ('d_vnat', [128, HTT*(H*65)], bf16), ('d_k0t', [128, DT], bf16),
                ('d_pT', [128, PTW], bf16), ('d_attn', [128, DT*TOK], bf16),
                ('d_pre1', [128, DT*TOK], f32), ('d_rden', [1, TOK], f32),
                ('d_rbs', [64, TOK], f32), ('d_den', [1, TOK], f32)):
            dbg[nm] = nc.declare_dram_parameter(nm, shape, dt, isOutput=True)

    id_f32 = nc.inline_tensor(np.eye(128, dtype=np.float32), name='id_f32')
    id_bf = nc.inline_tensor(np.eye(128, dtype=np.float32).astype(bfloat16), name='id_bf')
    ones_row_np = np.ones((1, 128), dtype=np.float32)
    ones_col_np = np.ones((128, 1), dtype=np.float32)
    onesr_f = nc.inline_tensor(ones_row_np, name='onesr_f')
    onesr_b = nc.inline_tensor(ones_row_np.astype(bfloat16), name='onesr_b')
    onesc_f = nc.inline_tensor(ones_col_np, name='onesc_f')

    with tile.TileContext(nc) as tc:
        ctx = contextlib.ExitStack()
        with ctx:
            st = ctx.enter_context(tc.tile_pool(name='state', bufs=1))
            wsm = ctx.enter_context(tc.tile_pool(name='wsm', bufs=6))
            wnat = ctx.enter_context(tc.tile_pool(name='wnat', bufs=1))
            w2p = ctx.enter_context(tc.tile_pool(name='w2p', bufs=2))
            ptp = ctx.enter_context(tc.tile_pool(name='ptp', bufs=2))
            bp = ctx.enter_context(tc.tile_pool(name='bias', bufs=8))
            bvp = ctx.enter_context(tc.tile_pool(name='bvp', bufs=2))
            sp = ctx.enter_context(tc.tile_pool(name='small', bufs=1))
            hd = ctx.enter_context(tc.tile_pool(name='hd', bufs=2))
            sqp = ctx.enter_context(tc.tile_pool(name='sq', bufs=2))
            lnb = ctx.enter_context(tc.tile_pool(name='lnb', bufs=1))
            rbp = ctx.enter_context(tc.tile_pool(name='rbp', bufs=2))
            ps_proj = ctx.enter_context(tc.tile_pool(name='ps_proj', bufs=2, space='PSUM'))
            ps_st = ctx.enter_context(tc.tile_pool(name='ps_st', bufs=3, space='PSUM'))
            ps_pv = ctx.enter_context(tc.tile_pool(name='ps_pv', bufs=2, space='PSUM'))
            ps_misc = ctx.enter_context(tc.tile_pool(name='ps_misc', bufs=1, space='PSUM'))
            dram = ctx.enter_context(tc.tile_pool(name='dram', bufs=2, space='DRAM'))

            # ---------------- persistent state tiles ----------------
            xT = st.tile([128, DT * TOK], f32, tag='xT')
            xhT = st.tile([128, DT * HT], bf16, tag='xhT')
            kT = st.tile([128, DT * HT], bf16, tag='kT')
            vnat = st.tile([128, HTT * (H * 65)], bf16, tag='vnat')
            vgnat = st.tile([128, TT * 769], bf16, tag='vgnat')
            bandM_sb = st.tile([128, PTW], bf16, tag='bandM')
            hT = st.tile([128, FT * TOK], bf16, tag='hT')
            # overlays: disjoint lifetimes within a layer share SBUF
            qT = hT[:, 0:DT * TOK]                     # q dies before W1 writes
            kgT = hT[:, DT * TOK:2 * DT * TOK]         # kg dies before W1 writes
            ab = st.tile([128, DT * TOK], bf16, tag='ab')
            attn_outT = ab                             # attn dies at Wo; then LN1 out
            xmidb = ab
            idsb = st.tile([128, 2 * DT], i32, tag='idsb')
            amb_sb = st.tile([128, TT], f32, tag='amb')
            selm_sb = st.tile([128, DT], bf16, tag='selm')
            idf_sb = st.tile([128, 128], f32, tag='idf')
            idb_sb = st.tile([128, 128], bf16, tag='idb')
            onesr_fs = st.tile([1, 128], f32, tag='onesr_f')
            onesr_bs = st.tile([1, 128], bf16, tag='onesr_b')
            onesc_fs = st.tile([128, 1], f32, tag='onesc_f')
            x0t = st.tile([128, DT], bf16, tag='x0t')
            k0t = st.tile([128, DT], bf16, tag='k0t')
            v0row = st.tile([1, H * 65], bf16, tag='v0row')
            qgblk = st.tile([128, DT * H], bf16, tag='qgblk')
            pgT = st.tile([128, TT * H], bf16, tag='pgT')
            ogs = st.tile([12, 65], f32, tag='ogs')
            ogr = st.tile([12, 1], f32, tag='ogr')
            ogbf = st.tile([12, 64], bf16, tag='ogbf')
            ogT6 = st.tile([128, DT], bf16, tag='ogT6')
            d0col = st.tile([128, DT], bf16, tag='d0col')
            if debug:
                dbg_den_sb = st.tile([1, TOK], f32, tag='dbgden', name='dbg_den_sb')
            else:
                dbg_den_sb = None
            eps_t = st.tile([128, 1], f32, tag='eps')
            nc.vector.memset(eps_t[:], 1e-5)
            nc.vector.memset(qgblk[:], 0.0)

            dma = nc.sync.dma_start
            # one-time loads
            dma(out=idf_sb[:], in_=id_f32[:, :])
            dma(out=idb_sb[:], in_=id_bf[:, :])
            dma(out=onesr_fs[:], in_=onesr_f[:, :])
            dma(out=onesr_bs[:], in_=onesr_b[:, :])
            dma(out=onesc_fs[:], in_=onesc_f[:, :])
            dma(out=bandM_sb[:], in_=P['bandM'][:, :])
            dma(out=idsb[:, 0:DT], in_=P['lidx'][:, :])
            dma(out=idsb[:, DT:2*DT], in_=P['ridx'][:, :])
            dma(out=amb_sb[:], in_=P['amb'][:, :])
            dma(out=selm_sb[:], in_=P['selm'][:, :])

            # ---------------- embedding: LN then transpose (gamma/beta post-T) ----------------
            eg_t = bp.tile([128, DT], f32, tag='bqkv')
            dma(out=eg_t[:], in_=P['embg'][:, :])
            eb_t = bp.tile([128, DT], f32, tag='bqkv')
            dma(out=eb_t[:], in_=P['embb'][:, :])
            for t in range(TT):
                xe = sqp.tile([128, D], f32, tag='cen')
                dma(out=xe[:], in_=P['emb'][t*128:(t+1)*128, :])
                mu = sp.tile([128, 1], f32, tag='mu_e')
                nc.vector.reduce_sum(out=mu[:], in_=xe[:], axis=mybir.AxisListType.X)
                nc.scalar.mul(out=mu[:], in_=mu[:], mul=1.0/D)
                nc.vector.tensor_scalar_sub(out=xe[:], in0=xe[:], scalar1=mu[:, 0:1])
                sq = sqp.tile([128, D], f32, tag='cen')
                nc.scalar.square(out=sq[:], in_=xe[:])
                var = sp.tile([128, 1], f32, tag='mu_e')
                nc.vector.reduce_sum(out=var[:], in_=sq[:], axis=mybir.AxisListType.X)
                nc.scalar.activation(out=var[:], in_=var[:], func=AF.Sqrt,
                                     bias=eps_t[:, 0:1], scale=1.0/D)
                nc.vector.reciprocal(out=var[:], in_=var[:])
                nc.vector.tensor_scalar_mul(out=xe[:], in0=xe[:], scalar1=var[:, 0:1])
                for di in range(DT):
                    pt = ps_misc.tile([128, 128], f32, tag='m1')
                    nc.tensor.transpose(out=pt[:], in_=xe[:, di*128:(di+1)*128],
                                        identity=idf_sb[:])
                    nc.vector.tensor_scalar(
                        out=xT[:, di*TOK + t*128 : di*TOK + (t+1)*128], in0=pt[:],
                        scalar1=eg_t[:, di:di+1], scalar2=eb_t[:, di:di+1],
                        op0=OP.mult, op1=OP.add)
                    nc.scalar.copy(out=xhT[:, di*HT + BLK + t*128 : di*HT + BLK + (t+1)*128],
                                   in_=xT[:, di*TOK + t*128 : di*TOK + (t+1)*128])

            # ---------------- helpers ----------------
            def load_w6(wname, l, do, tag):
                wt = wsm.tile([128, DT * 128], bf16, tag=tag)
                dma(out=wt[:], in_=P[wname][l, do, :, :])
                return wt

            def load_bias_cols(pname, l, n, tag):
                bt = bp.tile([128, n], f32, tag=tag)
                dma(out=bt[:], in_=P[pname][l, :, :])
                return bt

            def layer_norm(l, src, dstf, dstb_fn, gname, bname):
                g_t = load_bias_cols(gname, l, DT, 'g_ln')
                b_t = load_bias_cols(bname, l, DT, 'b_ln')
                s1 = ps_misc.tile([1, TOK], f32, tag='m1')
                s2 = ps_pv.tile([1, TOK], f32, tag='pv')
                for di in range(DT):
                    nc.tensor.matmul(s1[:], lhsT=onesc_fs[:, :],
                                     rhs=src[:, di*TOK:(di+1)*TOK],
                                     start=(di == 0), stop=(di == DT-1))
                    sq = sqp.tile([128, TOK], f32, tag='sq')
                    nc.scalar.square(out=sq[:], in_=src[:, di*TOK:(di+1)*TOK])
                    nc.tensor.matmul(s2[:], lhsT=onesc_fs[:, :], rhs=sq[:],
                                     start=(di == 0), stop=(di == DT-1))
                mu = sp.tile([1, TOK], f32, tag='ln_mu')
                ex2 = sp.tile([1, TOK], f32, tag='ln_ex2')
                nc.scalar.mul(out=mu[:], in_=s1[:], mul=1.0/D)
                nc.scalar.mul(out=ex2[:], in_=s2[:], mul=1.0/D)
                var = sp.tile([1, TOK], f32, tag='ln_var')
                nc.vector.tensor_mul(out=var[:], in0=mu[:], in1=mu[:])
                nc.vector.tensor_sub(out=var[:], in0=ex2[:], in1=var[:])
                nc.scalar.activation(out=var[:], in_=var[:], func=AF.Sqrt,
                                     bias=eps_t[0:1, 0:1])
                rsr = sp.tile([1, TOK], f32, tag='ln_rsr')
                scr = sp.tile([1, TOK], f32, tag='ln_scr')
                nc.vector.reciprocal_approx_accurate(out=rsr[:], in_=var[:], scratch=scr[:])
                mub = lnb.tile([128, TOK], f32, tag='mub')
                rsb = lnb.tile([128, TOK], f32, tag='rsb')
                nc.gpsimd.partition_broadcast(mub[:], mu[:])
                nc.gpsimd.partition_broadcast(rsb[:], rsr[:])
                for di in range(DT):
                    cen = sqp.tile([128, TOK], f32, tag='cen')
                    nc.vector.tensor_sub(out=cen[:], in0=src[:, di*TOK:(di+1)*TOK], in1=mub[:])
                    nc.vector.tensor_mul(out=cen[:], in0=cen[:], in1=rsb[:])
                    nc.vector.tensor_scalar(
                        out=dstf[:, di*TOK:(di+1)*TOK], in0=cen[:],
                        scalar1=g_t[:, di:di+1], scalar2=b_t[:, di:di+1],
                        op0=OP.mult, op1=OP.add)
                    nc.vector.tensor_copy(dstb_fn(di), in_=dstf[:, di*TOK:(di+1)*TOK])

            # ---------------- layers ----------------
            for l in range(n_layers):
                # ---- AllGather of residual stream (fires first) ----
                ag_in = dram.tile([D, TOK], bf16, tag='ag_in')
                ag_out = dram.tile([NCORE * D, TOK], bf16, tag='ag_out')
                for di in range(DT):
                    dma(out=ag_in[di*128:(di+1)*128, :],
                        in_=xhT[:, di*HT + BLK : di*HT + BLK + TOK])
                nc.gpsimd.collective_compute(
                    'AllGather', OP.bypass,
                    replica_groups=[list(range(NCORE))],
                    ins=[ag_in[:].opt()], outs=[ag_out[:].opt()])

                bq_t = load_bias_cols('bq', l, DT, 'bqkv')
                bk_t = load_bias_cols('bk', l, DT, 'bqkv')
                bqg_t = load_bias_cols('bqg', l, DT, 'bqkv')
                bkg_t = load_bias_cols('bkg', l, DT, 'bqkv')
                bo_t = load_bias_cols('bo', l, DT, 'bqkv')
                b2_t = load_bias_cols('b2', l, DT, 'bqkv')
                b1_t = load_bias_cols('b1', l, FT, 'b1')
                bvrow_t = bvp.tile([1, H * 65], bf16, tag='bvrow')
                dma(out=bvrow_t[:], in_=P['bvrow'][l, :].rearrange('(o d) -> o d', o=1))
                bvgrow_t = bvp.tile([1, 769], bf16, tag='bvgrow')
                dma(out=bvgrow_t[:], in_=P['bvgrow'][l, :].rearrange('(o d) -> o d', o=1))

                # ---- own-token projections (overlap with AllGather) ----
                def proj_chunks(wname, l, dst, bias_t, chunks, dst_span, evac):
                    # chunks: list of (src_off_in_halo, width, dst_off)
                    for do in range(DT):
                        wt = load_w6(wname, l, do, 'w_sm')
                        for (soff, w, doff) in chunks:
                            pp = ps_proj.tile([128, 512], f32, tag='pp')
                            for di in range(DT):
                                nc.tensor.matmul(
                                    pp[:, :w],
                                    lhsT=wt[:, di*128:(di+1)*128],
                                    rhs=xhT[:, di*HT + soff : di*HT + soff + w],
                                    start=(di == 0), stop=(di == DT-1))
                            evac(dst, do, pp, bias_t, w, doff, dst_span)

                def evac_dve(dst, do, pp, bias_t, w, doff, span):
                    nc.vector.tensor_scalar_add(
                        out=dst[:, do*span + doff : do*span + doff + w],
                        in0=pp[:, :w], scalar1=bias_t[:, do:do+1])

                def evac_act(dst, do, pp, bias_t, w, doff, span):
                    nc.scalar.activation(
                        out=dst[:, do*span + doff : do*span + doff + w], in_=pp[:, :w],
                        func=AF.Identity, bias=bias_t[:, do:do+1])

                # q (scale folded into weights+bias on host), kg: own tokens
                proj_chunks('Wqt', l, qT, bq_t, [(BLK, TOK, 0)], TOK, evac_dve)
                proj_chunks('Wkgt', l, kgT, bkg_t, [(BLK, TOK, 0)], TOK, evac_dve)

                # k own tokens
                for do in range(DT):
                    wt = load_w6('Wkt', l, do, 'w_sm')
                    pp = ps_proj.tile([128, 512], f32, tag='pp')
                    for di in range(DT):
                        nc.tensor.matmul(pp[:], lhsT=wt[:, di*128:(di+1)*128],
                                         rhs=xhT[:, di*HT + BLK : di*HT + BLK + TOK],
                                         start=(di == 0), stop=(di == DT-1))
                    nc.scalar.activation(out=kT[:, do*HT + BLK : do*HT + BLK + TOK],
                                         in_=pp[:], func=AF.Identity, bias=bk_t[:, do:do+1])

                # v own tiles (natural layout)
                wv_t = [wnat.tile([128, H * 65], bf16, tag='wv' + str(di), name=f'wv_{l}_{di}') for di in range(DT)]
                for di in range(DT):
                    dma(out=wv_t[di][:], in_=P['Wvnat'][l, di, :, :])
                def vnat_tiles(trange):
                    for t in trange:
                        for n0 in range(0, H * 65, 390):
                            n1 = n0 + 390
                            pp = ps_proj.tile([128, 512], f32, tag='pp')
                            for di in range(DT):
                                nc.tensor.matmul(
                                    pp[:, :390],
                                    lhsT=xhT[:, di*HT + t*128 : di*HT + (t+1)*128],
                                    rhs=wv_t[di][:, n0:n1], start=(di == 0), stop=False)
                            nc.tensor.matmul(pp[:, :390], lhsT=onesr_bs[:, :],
                                             rhs=bvrow_t[:, n0:n1], start=False, stop=True)
                            nc.vector.tensor_copy(vnat[:, t*(H*65) + n0 : t*(H*65) + n1],
                                                  in_=pp[:, :390])
                vnat_tiles([2, 3, 4, 5])

                # vg (global-v, own tokens)
                wvg_t = [wnat.tile([128, 769], bf16, tag='wv' + str(di) + 'g', name=f'wvg_{l}_{di}') for di in range(DT)]
                for di in range(DT):
                    dma(out=wvg_t[di][:], in_=P['Wvgnat'][l, di, :, :])
                for t in range(TT):
                    for n0 in range(0, 769, 512):
                        n1 = min(n0 + 512, 769)
                        pp = ps_proj.tile([128, 512], f32, tag='pp')
                        for di in range(DT):
                            nc.tensor.matmul(
                                pp[:, :n1-n0],
                                lhsT=xhT[:, di*HT + BLK + t*128 : di*HT + BLK + (t+1)*128],
                                rhs=wvg_t[di][:, n0:n1], start=(di == 0), stop=False)
                        nc.tensor.matmul(pp[:, :n1-n0], lhsT=onesr_bs[:, :],
                                         rhs=bvgrow_t[:, n0:n1], start=False, stop=True)
                        nc.vector.tensor_copy(vgnat[:, t*769 + n0 : t*769 + n1],
                                              in_=pp[:, :n1-n0])

                # ---- halo arrival: indirect gather + x0 ----
                for di in range(DT):
                    nc.gpsimd.indirect_dma_start(
                        out=xhT[:, di*HT : di*HT + BLK], out_offset=None,
                        in_=ag_out[:],
                        in_offset=bass.IndirectOffsetOnAxis(ap=idsb[:, di:di+1], axis=0),
                        element_offset=TOK - BLK)
                    nc.gpsimd.indirect_dma_start(
                        out=xhT[:, di*HT + BLK + TOK : (di+1)*HT], out_offset=None,
                        in_=ag_out[:],
                        in_offset=bass.IndirectOffsetOnAxis(ap=idsb[:, DT+di:DT+di+1], axis=0),
                        element_offset=0)
                for di in range(DT):
                    dma(out=x0t[:, di:di+1], in_=ag_out[di*128:(di+1)*128, 0:1])

                # k halo chunks + k0 rows (x0 stationary, reuses loaded Wk tile)
                k0row = sp.tile([1, D], f32, tag='k0row')
                qg0row = sp.tile([1, D], f32, tag='qg0row')
                for do in range(DT):
                    wt = load_w6('Wkt', l, do, 'w_sm')
                    for (soff, w) in ((0, BLK), (BLK + TOK, BLK)):
                        pp = ps_proj.tile([128, 512], f32, tag='pp')
                        for di in range(DT):
                            nc.tensor.matmul(pp[:, :w],
                                             lhsT=wt[:, di*128:(di+1)*128],
                                             rhs=xhT[:, di*HT + soff : di*HT + soff + w],
                                             start=(di == 0), stop=(di == DT-1))
                        nc.scalar.activation(out=kT[:, do*HT + soff : do*HT + soff + w],
                                             in_=pp[:, :w], func=AF.Identity,
                                             bias=bk_t[:, do:do+1])
                    pk = ps_misc.tile([1, 128], f32, tag='m1')
                    for di in range(DT):
                        nc.tensor.matmul(pk[:], lhsT=x0t[:, di:di+1],
                                         rhs=wt[:, di*128:(di+1)*128],
                                         start=(di == 0), stop=(di == DT-1))
                    nc.vector.tensor_copy(k0row[:, do*128:(do+1)*128], in_=pk[:])
                vnat_tiles([0, 1, 6, 7])

                for do in range(DT):
                    wqg = load_w6('Wqgt', l, do, 'w_sm')
                    pq = ps_misc.tile([1, 128], f32, tag='m1')
                    for di in range(DT):
                        nc.tensor.matmul(pq[:], lhsT=x0t[:, di:di+1],
                                         rhs=wqg[:, di*128:(di+1)*128],
                                         start=(di == 0), stop=(di == DT-1))
                    nc.vector.tensor_copy(qg0row[:, do*128:(do+1)*128], in_=pq[:])
                # roundtrip through DRAM to transpose rows into column layout
                k0d = dram.tile([1, D], f32, tag='k0d')
                qg0d = dram.tile([1, D], f32, tag='qg0d')
                dma(out=k0d[:, :], in_=k0row[:, :])
                dma(out=qg0d[:, :], in_=qg0row[:, :])
                k0c = sp.tile([128, DT], f32, tag='k0c')
                qg0c = sp.tile([128, DT], f32, tag='qg0c')
                dma(out=k0c[:, :], in_=k0d[0:1, :].rearrange('o (t p) -> p (o t)', p=128))
                dma(out=qg0c[:, :], in_=qg0d[0:1, :].rearrange('o (t p) -> p (o t)', p=128))
                nc.vector.tensor_add(out=k0t[:, :], in0=k0c[:, :], in1=bk_t[:, 0:DT])
                qgcol = sp.tile([128, DT], bf16, tag='qgcol')
                nc.vector.tensor_add(out=qgcol[:, :], in0=qg0c[:, :], in1=bqg_t[:, 0:DT])
                for h in range(H):
                    nc.vector.tensor_copy(
                        qgblk[(h % 2)*64:(h % 2)*64+64, (h//2)*H + h : (h//2)*H + h + 1],
                        in_=qgcol[(h % 2)*64:(h % 2)*64+64, h//2:h//2+1])
                # v0 row directly from Wvnat (token-0 column as lhsT)
                for n0 in range(0, H * 65, 390):
                    n1 = n0 + 390
                    pv0 = ps_misc.tile([1, 512], f32, tag='m1')
                    for di in range(DT):
                        nc.tensor.matmul(pv0[:, :390], lhsT=x0t[:, di:di+1],
                                         rhs=wv_t[di][:, n0:n1], start=(di == 0), stop=False)
                    nc.tensor.matmul(pv0[:, :390], lhsT=onesr_bs[:, 0:1],
                                     rhs=bvrow_t[:, n0:n1], start=False, stop=True)
                    nc.vector.tensor_copy(v0row[:, n0:n1], in_=pv0[:, :390])

                # ---- global row: sg over own tokens, AllReduce early ----
                for t in range(TT):
                    sgt = ps_misc.tile([128, H], f32, tag='m1')
                    for di in range(DT):
                        nc.tensor.matmul(sgt[:],
                                         lhsT=kgT[:, di*TOK + t*128 : di*TOK + (t+1)*128],
                                         rhs=qgblk[:, di*H:(di+1)*H],
                                         start=(di == 0), stop=(di == DT-1))
                    nc.scalar.activation(out=pgT[:, t*H:(t+1)*H], in_=sgt[:], func=AF.Exp,
                                         bias=amb_sb[:, t:t+1], scale=1.0)
                oga = ps_pv.tile([12, 512], f32, tag='pv')
                ogb = ps_pv.tile([12, 257], f32, tag='pv')
                for t in range(TT):
                    nc.tensor.matmul(oga[:], lhsT=pgT[:, t*H:(t+1)*H],
                                     rhs=vgnat[:, t*769 : t*769 + 512],
                                     start=(t == 0), stop=(t == TT-1))
                for t in range(TT):
                    nc.tensor.matmul(ogb[:], lhsT=pgT[:, t*H:(t+1)*H],
                                     rhs=vgnat[:, t*769 + 512 : (t+1)*769],
                                     start=(t == 0), stop=(t == TT-1))
                ogsb_a = sqp.tile([12, 512], f32, tag='sq')
                ogsb_b = sqp.tile([12, 257], f32, tag='sq')
                nc.vector.tensor_copy(ogsb_a[:], in_=oga[:])
                nc.vector.tensor_copy(ogsb_b[:], in_=ogb[:])
                ar_in = dram.tile([12, 65], f32, tag='ar_in')
                ar_out = dram.tile([12, 65], f32, tag='ar_out')
                for h in range(H):
                    if h < 8:
                        dma(out=ar_in[h:h+1, 0:64], in_=ogsb_a[h:h+1, h*64:(h+1)*64])
                    else:
                        dma(out=ar_in[h:h+1, 0:64],
                            in_=ogsb_b[h:h+1, h*64-512:(h+1)*64-512])
                dma(out=ar_in[:, 64:65], in_=ogsb_b[:, 256:257])
                nc.gpsimd.collective_compute(
                    'AllReduce', OP.add, replica_groups=[list(range(NCORE))],
                    ins=[ar_in[:].opt()], outs=[ar_out[:].opt()])
                dma(out=ogs[:], in_=ar_out[:])
                # og post-processing (normalize + transpose to column layout)
                ogscr = sp.tile([12, 1], f32, tag='ogscr')
                nc.vector.reciprocal_approx_accurate(out=ogr[:], in_=ogs[:, 64:65],
                                                     scratch=ogscr[:])
                nc.vector.tensor_scalar_mul(out=ogbf[:], in0=ogs[:, 0:64],
                                            scalar1=ogr[:, 0:1])
                ogt_ps = ps_misc.tile([64, 12], bf16, tag='m1')
                nc.tensor.transpose(out=ogt_ps[:], in_=ogbf[:], identity=idb_sb[0:12, 0:12])
                ogT = sp.tile([64, 12], bf16, tag='ogT')
                nc.vector.tensor_copy(ogT[:], in_=ogt_ps[:])
                for h in range(H):
                    dma(out=ogT6[(h % 2)*64:(h % 2)*64+64, h//2:h//2+1],
                        in_=ogT[:, h:h+1])

                if debug and l == 0:
                    dma(out=dbg['d_qT'][:, :], in_=qT[:, :])
                    dma(out=dbg['d_kT'][:, :], in_=kT[:])
                    dma(out=dbg['d_vnat'][:, :], in_=vnat[:])
                    dma(out=dbg['d_k0t'][:, :], in_=k0t[:])
                # ---- band attention per head (live query ranges) ----
                for h in range(H):
                    po = (h % 2) * 64
                    dt_ = h // 2
                    pT = ptp.tile([128, PTW], bf16, tag='pT')
                    sg = ps_misc.tile([1, TOK], f32, tag='m1')
                    nc.tensor.matmul(sg[:], lhsT=k0t[po:po+64, dt_:dt_+1],
                                     rhs=qT[po:po+64, dt_*TOK:(dt_+1)*TOK],
                                     start=True, stop=True)
                    pglob = hd.tile([1, TOK], bf16, tag='pglob')
                    nc.scalar.activation(out=pglob[:], in_=sg[:], func=AF.Exp)
                    for c in CORDER:
                        lv = LIVE[c]; ql = QLO[c]; of = OFF[c]
                        stp = ps_st.tile([128, 512], f32, tag='st')
                        nc.tensor.matmul(
                            stp[:, :lv],
                            lhsT=kT[po:po+64, dt_*HT + c*128 : dt_*HT + (c+1)*128],
                            rhs=qT[po:po+64, dt_*TOK + ql : dt_*TOK + ql + lv],
                            start=True, stop=True)
                        nc.scalar.activation(out=pT[:, of:of+lv], in_=stp[:, :lv],
                                             func=AF.Exp)
                        nc.vector.tensor_mul(out=pT[:, of:of+lv],
                                             in0=pT[:, of:of+lv],
                                             in1=bandM_sb[:, of:of+lv])
                    pvp = ps_pv.tile([65, TOK], f32, tag='pv')
                    for i, c in enumerate(CORDER):
                        lv = LIVE[c]; ql = QLO[c]; of = OFF[c]
                        nc.tensor.matmul(
                            pvp[:, ql:ql+lv],
                            lhsT=vnat[:, c*(H*65) + h*65 : c*(H*65) + h*65 + 65],
                            rhs=pT[:, of:of+lv], start=(i == 0), stop=False)
                    nc.tensor.matmul(pvp[:], lhsT=v0row[:, h*65:h*65+65], rhs=pglob[:],
                                     start=False, stop=True)
                    den_sb = hd.tile([1, TOK], f32, tag='den_sb')
                    nc.vector.tensor_copy(den_sb[:], in_=pvp[64:65, :])
                    rden = hd.tile([1, TOK], f32, tag='rden')
                    nc.vector.reciprocal_approx_fast(out=rden[:], in_=den_sb[:])
                    rbs = rbp.tile([64, TOK], f32, tag='rbs')
                    if debug and l == 0 and h == 0:
                        dma(out=dbg['d_rden'][:, :], in_=rden[:])
                        nc.vector.tensor_copy(dbg_den_sb[:], in_=pvp[64:65, :])
                        dma(out=dbg['d_den'][:, :], in_=dbg_den_sb[:])
                    nc.gpsimd.partition_broadcast(rbs[:], rden[:])
                    if debug and l == 0 and h == 0:
                        dma(out=dbg['d_rbs'][:, :], in_=rbs[:])
                    nc.vector.tensor_mul(out=attn_outT[po:po+64, dt_*TOK:(dt_+1)*TOK],
                                         in0=pvp[0:64, :], in1=rbs[:])
                    if debug and l == 0 and h == 0:
                        dma(out=dbg['d_pT'][:, :], in_=pT[:])

                # ---- token-0 column delta (global-token output correction) ----
                if debug and l == 0:
                    dma(out=dbg['d_attn'][:, :], in_=attn_outT[:, :])
                a0 = sp.tile([128, DT], bf16, tag='a0')
                for di in range(DT):
                    nc.vector.tensor_copy(a0[:, di:di+1], in_=attn_outT[:, di*TOK:di*TOK+1])
                nc.vector.tensor_sub(out=d0col[:, :], in0=ogT6[:, :], in1=a0[:, :])
                nc.vector.tensor_mul(out=d0col[:, :], in0=d0col[:, :], in1=selm_sb[:, :])

                # ---- output projection + residual (+ token-0 delta) ----
                for do in range(DT):
                    wt = load_w6('Wot', l, do, 'w_sm')
                    pp = ps_proj.tile([128, 512], f32, tag='pp')
                    for di in range(DT):
                        nc.tensor.matmul(pp[:], lhsT=wt[:, di*128:(di+1)*128],
                                         rhs=attn_outT[:, di*TOK:(di+1)*TOK],
                                         start=(di == 0), stop=(di == DT-1))
                    pp0 = ps_misc.tile([128, 1], f32, tag='m1')
                    for di in range(DT):
                        nc.tensor.matmul(pp0[:], lhsT=wt[:, di*128:(di+1)*128],
                                         rhs=d0col[:, di:di+1],
                                         start=(di == 0), stop=(di == DT-1))
                    nc.vector.scalar_tensor_tensor(
                        out=xT[:, do*TOK:(do+1)*TOK], in0=pp[:], scalar=bo_t[:, do:do+1],
                        in1=xT[:, do*TOK:(do+1)*TOK], op0=OP.add, op1=OP.add)
                    nc.vector.tensor_add(out=xT[:, do*TOK:do*TOK+1],
                                         in0=xT[:, do*TOK:do*TOK+1], in1=pp0[:])

                if debug and l == 0:
                    dma(out=dbg['d_pre1'][:, :], in_=xT[:])
                layer_norm(l, xT, xT, lambda di: xmidb[:, di*TOK:(di+1)*TOK],
                           'ln1g', 'ln1b')

                # ---- FFN ----
                for do in range(FT):
                    wt = load_w6('W1t', l, do, 'w_sm')
                    pp = ps_proj.tile([128, 512], f32, tag='pp')
                    for di in range(DT):
                        nc.tensor.matmul(pp[:], lhsT=wt[:, di*128:(di+1)*128],
                                         rhs=xmidb[:, di*TOK:(di+1)*TOK],
                                         start=(di == 0), stop=(di == DT-1))
                    nc.scalar.activation(out=hT[:, do*TOK:(do+1)*TOK], in_=pp[:],
                                         func=AF.Gelu, bias=b1_t[:, do:do+1])
                for do in range(DT):
                    wt = w2p.tile([128, FT * 128], bf16, tag='w2')
                    dma(out=wt[:], in_=P['W2t'][l, do, :, :])
                    pp = ps_proj.tile([128, 512], f32, tag='pp')
                    for di in range(FT):
                        nc.tensor.matmul(pp[:], lhsT=wt[:, di*128:(di+1)*128],
                                         rhs=hT[:, di*TOK:(di+1)*TOK],
                                         start=(di == 0), stop=(di == FT-1))
                    nc.vector.scalar_tensor_tensor(
                        out=xT[:, do*TOK:(do+1)*TOK], in0=pp[:], scalar=b2_t[:, do:do+1],
                        in1=xT[:, do*TOK:(do+1)*TOK], op0=OP.add, op1=OP.add)

                layer_norm(l, xT, xT,
                           lambda di: xhT[:, di*HT + BLK : di*HT + BLK + TOK],
                           'ln2g', 'ln2b')

            # ---------------- head (token 0; real on core 0) ----------------
            x0f = sp.tile([128, DT], bf16, tag='x0f')
            for di in range(DT):
                nc.scalar.copy(out=x0f[:, di:di+1], in_=xT[:, di*TOK:di*TOK+1])
            bd_t = bp.tile([128, DT], f32, tag='bqkv')
            dma(out=bd_t[:], in_=P['bd'][:, :])
            clsT = sp.tile([128, DT], bf16, tag='clsT')
            for do in range(DT):
                wt = wsm.tile([128, DT * 128], bf16, tag='w_sm')
                dma(out=wt[:], in_=P['Wdt'][do, :, :])
                pp = ps_misc.tile([128, 1], f32, tag='m1')
                for di in range(DT):
                    nc.tensor.matmul(pp[:], lhsT=wt[:, di*128:(di+1)*128],
                                     rhs=x0f[:, di:di+1], start=(di == 0), stop=(di == DT-1))
                nc.scalar.activation(out=clsT[:, do:do+1], in_=pp[:], func=AF.Tanh,
                                     bias=bd_t[:, do:do+1])
            wcls = sp.tile([128, DT*2], bf16, tag='wcls')
            dma(out=wcls[:], in_=P['Wclst'][:, :, :].rearrange('t p c -> p t c'))
            lgp = ps_misc.tile([2, 1], f32, tag='m1')
            for di in range(DT):
                nc.tensor.matmul(lgp[:], lhsT=wcls[:, di*2:(di+1)*2], rhs=clsT[:, di:di+1],
                                 start=(di == 0), stop=(di == DT-1))
            bcls_t = sp.tile([2, 1], f32, tag='bcls')
            dma(out=bcls_t[:], in_=P['bcls'].ap().rearrange('(c o) -> c o', o=1))
            lgT = sp.tile([2, 1], f32, tag='lgT')
            nc.vector.tensor_add(out=lgT[:], in0=lgp[:], in1=bcls_t[:])
            lg_ps = ps_misc.tile([1, 2], f32, tag='m1')
            nc.tensor.matmul(lg_ps[:], lhsT=lgT[:], rhs=idf_sb[0:2, 0:2], start=True, stop=True)
            lg = sp.tile([1, 2], f32, tag='lg')
            nc.scalar.copy(out=lg[:], in_=lg_ps[:])
            muh = sp.tile([1, 1], f32, tag='h_mu')
            nc.vector.reduce_sum(out=muh[:], in_=lg[:], axis=mybir.AxisListType.X)
            nc.scalar.mul(out=muh[:], in_=muh[:], mul=0.5)
            nc.vector.tensor_scalar_sub(out=lg[:], in0=lg[:], scalar1=muh[:, 0:1])
            sqh = sp.tile([1, 2], f32, tag='h_sq')
            nc.scalar.square(out=sqh[:], in_=lg[:])
            varh = sp.tile([1, 1], f32, tag='h_var')
            nc.vector.reduce_sum(out=varh[:], in_=sqh[:], axis=mybir.AxisListType.X)
            nc.scalar.activation(out=varh[:], in_=varh[:], func=AF.Sqrt,
                                 bias=eps_t[0:1, 0:1], scale=0.5)
            nc.vector.reciprocal(out=varh[:], in_=varh[:])
            nc.vector.tensor_scalar_mul(out=lg[:], in0=lg[:], scalar1=varh[:, 0:1])
            mg_t = sp.tile([1, 2], f32, tag='mg')
            dma(out=mg_t[:], in_=P['mg'].ap().rearrange('(o c) -> o c', o=1))
            mb_t = sp.tile([1, 2], f32, tag='mbt')
            dma(out=mb_t[:], in_=P['mb'].ap().rearrange('(o c) -> o c', o=1))
            nc.vector.tensor_mul(out=lg[:], in0=lg[:], in1=mg_t[:])
            nc.vector.tensor_add(out=lg[:], in0=lg[:], in1=mb_t[:])
            one1 = sp.tile([1, 1], f32, tag='one1')
            nc.vector.memset(one1[:], 1.0)
            yt_ps = ps_misc.tile([2, 1], f32, tag='m1')
            nc.tensor.matmul(yt_ps[:], lhsT=lg[:], rhs=one1[:], start=True, stop=True)
            yT = sp.tile([2, 1], bf16, tag='yT')
            nc.scalar.copy(out=yT[:], in_=yt_ps[:])
            whh = sp.tile([2, 128], bf16, tag='whh')
            dma(out=whh[:], in_=P['Whh'][:, :])
            h1p = ps_misc.tile([128, 1], f32, tag='m1')
            nc.tensor.matmul(h1p[:], lhsT=whh[:], rhs=yT[:], start=True, stop=True)
            bh_t = sp.tile([128, 1], f32, tag='bh')
            dma(out=bh_t[:], in_=P['bh'].ap().rearrange('(p o) -> p o', o=1))
            h1 = sp.tile([128, 1], bf16, tag='h1')
            nc.scalar.activation(out=h1[:], in_=h1p[:], func=AF.Relu, bias=bh_t[:, 0:1])
            wop = sp.tile([128, 2], bf16, tag='wop')
            dma(out=wop[:], in_=P['Wopt'][:, :])
            o2p = ps_misc.tile([2, 1], f32, tag='m1')
            nc.tensor.matmul(o2p[:], lhsT=wop[:], rhs=h1[:], start=True, stop=True)
            bop_t = sp.tile([2, 1], f32, tag='bop')
            dma(out=bop_t[:], in_=P['bop'].ap().rearrange('(c o) -> c o', o=1))
            o2 = sp.tile([2, 1], f32, tag='o2')
            nc.vector.tensor_add(out=o2[:], in0=o2p[:], in1=bop_t[:])
            dma(out=out_ext[:, :], in_=o2[:])
            dma(out=xout_ext[:, :], in_=xT[:])

    nc.finalize()
    return nc


def _host_prep(inputs, n_layers=L):
    f32 = np.float32
    ids = np.asarray(inputs['x_ids']).reshape(-1).astype(np.int64)
    amask = np.asarray(inputs['attention_mask']).reshape(-1)
    gmask = np.asarray(inputs['global_attention_mask']).reshape(-1)
    W = {k: np.asarray(v, dtype=f32) for k, v in inputs.items()
         if k not in ('x_ids', 'attention_mask', 'global_attention_mask')}

    c = np.arange(3*BLK); r = np.arange(BLK)
    band_ok = np.abs(c[None, :] - BLK - r[:, None]) <= BLK
    key_idx = np.arange(NB)[:, None]*BLK - BLK + c[None, :]
    in_rng = (key_idx >= 0) & (key_idx < S)
    safe = np.clip(key_idx, 0, S-1)
    key_ok = in_rng & (amask[safe] > 0) & (gmask[safe] <= 0)
    band_mask = band_ok[None, :, :] & key_ok[:, None, :]   # [NB, BLK(q), 3BLK(k)]

    emb_full = (W['word_emb'][ids] + W['pos_emb'][np.arange(S)+2]
                + W['type_emb']).astype(f32)

    def tiles_lhsT(w):   # [din, dout] -> [do, 128p, di*128] (partition-contiguous)
        din, dout = w.shape
        t = w.reshape(din//128, 128, dout//128, 128)
        return np.ascontiguousarray(t.transpose(2, 1, 0, 3).reshape(
            dout//128, 128, (din//128)*128)).astype(bfloat16)

    def stack_l(w, scale=1.0):
        if n_layers == 0:
            return np.zeros((0,), np.float32).astype(bfloat16)
        return np.stack([tiles_lhsT(w[l] * scale) for l in range(n_layers)])

    cm = {}
    cm['Wqt'] = stack_l(W['Wq'], SCALE); cm['Wkt'] = stack_l(W['Wk'])
    cm['Wqgt'] = stack_l(W['Wqg'], SCALE); cm['Wkgt'] = stack_l(W['Wkg'])
    cm['Wot'] = stack_l(W['Wo'])
    wv_ext = np.zeros((n_layers, DT, 128, H, 65), np.float32)
    wv_ext[:, :, :, :, 0:64] = W['Wv'][:n_layers].reshape(n_layers, DT, 128, H, 64)
    cm['Wvnat'] = np.ascontiguousarray(
        wv_ext.reshape(n_layers, DT, 128, H * 65)).astype(bfloat16)
    wvg_ext = np.zeros((n_layers, DT, 128, 769), np.float32)
    wvg_ext[:, :, :, 0:768] = W['Wvg'][:n_layers].reshape(n_layers, DT, 128, D)
    cm['Wvgnat'] = np.ascontiguousarray(wvg_ext).astype(bfloat16)
    cm['W1t'] = stack_l(W['W1']); cm['W2t'] = stack_l(W['W2'])
    def bcols(v, n):   # [L, n*128] -> [L, 128, n]
        return np.ascontiguousarray(
            v.reshape(n_layers, n, 128).transpose(0, 2, 1)).astype(f32)
    cm['bq'] = bcols(W['bq'][:n_layers] * SCALE, DT)
    cm['bqg'] = bcols(W['bqg'][:n_layers] * SCALE, DT)
    for b in ['bk', 'bkg', 'bo', 'b2']:
        cm[b] = bcols(W[b][:n_layers], DT)
    cm['b1'] = bcols(W['b1'][:n_layers], FT)
    bv_ext = np.zeros((n_layers, H, 65), np.float32)
    bv_ext[:, :, 0:64] = W['bv'][:n_layers].reshape(n_layers, H, 64)
    bv_ext[:, :, 64] = 1.0
    cm['bvrow'] = np.ascontiguousarray(bv_ext.reshape(n_layers, H*65)).astype(bfloat16)
    bvg_ext = np.zeros((n_layers, 769), np.float32)
    bvg_ext[:, 0:768] = W['bvg'][:n_layers]
    bvg_ext[:, 768] = 1.0
    cm['bvgrow'] = np.ascontiguousarray(bvg_ext).astype(bfloat16)
    cm['ln1g'] = bcols(W['ln1_g'][:n_layers], DT)
    cm['ln1b'] = bcols(W['ln1_b'][:n_layers], DT)
    cm['ln2g'] = bcols(W['ln2_g'][:n_layers], DT)
    cm['ln2b'] = bcols(W['ln2_b'][:n_layers], DT)
    cm['embg'] = np.ascontiguousarray(
        W['emb_ln_g'].reshape(DT, 128).T).astype(f32)
    cm['embb'] = np.ascontiguousarray(
        W['emb_ln_b'].reshape(DT, 128).T).astype(f32)
    cm['Wdt'] = tiles_lhsT(W['Wd'])
    cm['Wclst'] = np.ascontiguousarray(W['Wcls'].reshape(DT, 128, 2)).astype(bfloat16)
    cm['Whh'] = W['Wh'].astype(bfloat16)
    cm['Wopt'] = W['Wop'].astype(bfloat16)
    cm['bd'] = np.ascontiguousarray(W['bd'].reshape(DT, 128).T).astype(f32)
    cm['bcls'] = W['bcls'].astype(f32)
    sm = np.exp(W['mix_w'] - W['mix_w'].max()); sm = sm / sm.sum()
    cm['mg'] = (W['mln_g'] * sm.sum()).astype(f32)
    cm['mb'] = W['mln_b'].astype(f32)
    cm['bh'] = W['bh'].astype(f32)
    cm['bop'] = W['bop'].astype(f32)

    in_maps = []
    for core in range(NCORE):
        m = dict(cm)
        b0, b1_ = 2*core, 2*core + 1
        M = np.zeros((HT, TOK), f32)
        M[0:3*BLK, 0:BLK] = band_mask[b0].T
        M[BLK:BLK+3*BLK, BLK:2*BLK] = band_mask[b1_].T
        Mp = np.zeros((128, PTW), f32)
        for cc in range(HTT):
            blk = M[cc*128:(cc+1)*128, QLO[cc]:QLO[cc]+LIVE[cc]]
            Mp[:, OFF[cc]:OFF[cc]+LIVE[cc]] = blk
        m['bandM'] = Mp.astype(bfloat16)
        amb = np.where(amask[core*TOK:(core+1)*TOK] > 0, 0.0, NEG).astype(f32)
        m['amb'] = np.ascontiguousarray(amb.reshape(TT, 128).T).astype(f32)
        sel = np.full((128, DT), 1.0 if core == 0 else 0.0, f32)
        m['selm'] = sel.astype(bfloat16)
        left = max(core - 1, 0); right = min(core + 1, NCORE - 1)
        m['lidx'] = np.ascontiguousarray(
            (left*D + np.arange(D)).reshape(DT, 128).T).astype(np.int32)
        m['ridx'] = np.ascontiguousarray(
            (right*D + np.arange(D)).reshape(DT, 128).T).astype(np.int32)
        m['emb'] = emb_full[core*TOK:(core+1)*TOK]
        in_maps.append(m)
    return in_maps


_NC_CACHE = {}

def kernel(_n_layers=L, _want_hidden=False, _spmd_kwargs=None, _debug=False, **inputs):
    from concourse.bass_utils import run_bass_kernel_spmd
    n_layers = _n_layers
    key = (n_layers, _debug)
    if key not in _NC_CACHE:
        _NC_CACHE[key] = _build(n_layers, debug=_debug)
    nc = _NC_CACHE[key]
    in_maps = _host_prep(inputs, n_layers)
    res = run_bass_kernel_spmd(nc, in_maps, core_ids=list(range(NCORE)),
                               **(_spmd_kwargs or {}))
    out = np.asarray(res.results[0]['out']).reshape(1, 2)
    if _want_hidden:
        hid = np.concatenate(
            [np.asarray(res.results[c]['xout']).reshape(128, DT, TOK)
             .transpose(2, 1, 0).reshape(TOK, D) for c in range(NCORE)], axis=0)
        return out.astype(np.float32), hid, res
    return out.astype(np.float32)
